# revision 15
# baseline (speedup 1.0000x reference)
"""LIF spiking-neuron forward kernel for Trainium2 (8 NeuronCores, data-parallel
over neurons).

For x[B,N,T] and per-neuron params decay_m/decay_s/vth[N]:
    M_t = dm*(M_{t-1} + x_t);  S_t = ds*(S_{t-1} + x_t)
    E_t = dm*E_{t-1} + vth*o_{t-1}
    u_t = M_t - S_t - E_t - vth;  o_t = (u_t > 0)
returns the spike train o[B,N,T] (f32).

Per core (512 neurons = 4 chunks of 128 partitions).  Design notes:

  cascade:  M_t - S_t = (dm-ds) * H_t where H = scan_ds(scan_dm(x)) is the
            two-pole cascade of first-order scans (partial fractions of
            dm*z/((z-dm)(z-ds)) - ds*z/((z-dm)(z-ds)); the numerator
            difference is exactly (dm-ds)).  With the scan form
            state' = (state + x)*d the cascade yields dm*ds*H, so the host
            pre-scales x by c = (dm-ds)/(dm*ds*vth) per neuron, making
            r''_t = (M_t - S_t)/vth - 1 = cascade(c*x)_t - 1.
            This kills the former M-S subtract pass entirely and removes the
            per-partition 1/vth scale from the eviction.

  phase 1:  per b-batch (4 b's, all t): DMA x in, scan1 (dm) then scan2 (ds)
            in place on DVE (the HW scan runs at ~2.1ns/col = 2 cyc/elem,
            ~4.4us per 2064-col batch; this is the phase-1 floor), then one
            ScalarE activation evicts the batch into the big R tile with
            bias=-1 (layout (t, h, b), contiguous per t).  Evictions and DMA
            hide completely under the scans.

  phase 2:  normalized threshold recurrence (P = E/vth):
               o_t = (r''_t > P_t);  P_{t+1} = dm*P_t + o_t
            The chain is DVE-latency-bound (~213ns/instruction incl. drain),
            so it runs as TWO interLEAVED independent h-pair chains (P2HIL):
            per t: is_gt(h01), is_gt(h23), stt(h0), stt(h2), stt(h1), stt(h3)
            - every RAW link has an independent instruction between it, which
            overlaps the write-to-read pipeline drains (~1.28us/t vs 1.46us/t
            for the naive 5-instruction order).
            o is written as uint8 (exact 0/1) into a small per-t-block O tile
            and DMA'd out as uint8 (4x less output traffic); the stt P-update
            reads o back with uint8->f32 upcast.
            NOTE: the Pool engine only supports memset/copy/add/sub/mult/
            tensor_scalar (no comparisons/scans/stt - walrus rejects them),
            and is ~2x slower per column than DVE with ~250ns/instruction
            overhead, so phase 2 cannot be split across engines.

  host:     pre-scales/pads x into the scan layout and precomputes the scan
            decay tensors (dmcat/dscat with zero separator columns) so the
            device does no setup work; converts the uint8 spike train back to
            f32 on return.  Rounding differs from the reference by ~1e-6,
            flipping O(1) borderline spikes out of 33.5M (tolerance 2e-2).

  measured (HW, per-exec, 8 cores): phase 1 ~154us + phase 2 ~164us
  = ~299us total, vs 372us for the previous all-f32 subtract-based design.
"""

import numpy as np

import concourse.bacc as bacc
import concourse.bass as bass
import concourse.mybir as mybir
import concourse.tile as tile
from concourse.bass_utils import run_bass_kernel_spmd

F32 = mybir.dt.float32
U8 = mybir.dt.uint8
ALU = mybir.AluOpType
COPY = mybir.ActivationFunctionType.Copy

B, N, T = 64, 4096, 128
NCORES = 8
NLOC = N // NCORES          # 512 neurons per core
NH = NLOC // 128            # 4 neuron chunks of 128 (partition dim)
NB = 4                      # batch of b's per scan instruction
NBAT = B // NB              # 16 scan batches
NG = NB * NH                # 16 groups per scan batch, ordered h-major
TP = T + 1                  # per-group pitch in scan layout (sep column)
TBLK = 8                    # t-block size for the overlapped output DMA

# tunables (validated on HW).  NOTE: the Pool engine only supports
# memset/copy/add/sub/mult/tensor_scalar — no comparisons, no scans, no
# scalar_tensor_tensor — so phase 2 and the scans must stay on DVE.
PSPLIT = 4                          # chunks [0,PSPLIT) on DVE, rest on Pool
POOL_SCAN2 = frozenset()            # batches whose scan2 runs on Pool
P2MERGE = False                     # 3-instr/t phase 2 (wide tt) vs 5-instr
P2IL = 0                            # >0: interleave P2IL b-half chains
P2HIL = True                        # interleave 2 h-pair chains (stt updates)
WSCAN = 1                           # batches per scan instruction
SREORDER = False                    # software-pipeline scan1(i+1) before scan2(i)
POOL_EVICT = frozenset()            # batches evicted on Pool (ts add -1)


LAST_RESULTS = None

_cached_program = None


def build_program(rep: int = 1, psplit: int = PSPLIT,
                  pool_scan2=POOL_SCAN2, phases=(1, 2), tblk: int = TBLK,
                  hsep: bool = False, xbufs: int = 3,
                  p2merge: bool = P2MERGE, p2il: int = P2IL,
                  p2hil: bool = P2HIL, wscan: int = WSCAN,
                  sreorder: bool = SREORDER,
                  pool_evict=POOL_EVICT) -> bass.Bass:
    """rep=1 is the production kernel.  rep>1 wraps the whole computation in
    a hardware loop (tc.For_i) that re-runs it `rep` times per NEFF
    execution - used by test.py to amortize per-dispatch overhead out of the
    per-execution timing (each iteration redoes all DMA + compute)."""
    nc = bacc.Bacc(None, target_bir_lowering=False)
    # x pre-scaled by c[n] and pre-padded on host into the scan layout:
    # [128, NBAT, NG*TP], group g = h*NB + bl, b = i*NB + bl, n = h*128 + p.
    x_d = nc.declare_dram_parameter("x", [128, NBAT, NG * TP], F32, isOutput=False)
    dmcat_d = nc.declare_dram_parameter("dmcat", [128, NG * TP], F32, isOutput=False)
    dscat_d = nc.declare_dram_parameter("dscat", [128, NG * TP], F32, isOutput=False)
    dmc_d = nc.declare_dram_parameter("dmc", [128, NH], F32, isOutput=False)
    dmb_d = nc.declare_dram_parameter("dmb", [128, NH * B], F32, isOutput=False)
    # out[p, t*NH*B + h*B + b] = o[b, h*128+p, t] as uint8; host converts.
    out_d = nc.declare_dram_parameter("out", [128, T * NH * B], U8, isOutput=True)

    with tile.TileContext(nc) as tc:
        with (
            tc.tile_pool(name="big", bufs=1) as bigp,
            tc.tile_pool(name="xin", bufs=xbufs) as xp,
            tc.tile_pool(name="oout", bufs=2) as op,
            tc.tile_pool(name="const", bufs=1) as cp,
        ):
            # R: r'' in layout (t, h, b): f = t*(NH*B) + h*B + b
            R = bigp.tile([128, T * NH * B], F32)
            Rv = R[:].rearrange("p (t h b) -> p t h b", t=T, h=NH, b=B)

            dmCat = cp.tile([128, wscan * NG * TP], F32)
            dsCat = cp.tile([128, wscan * NG * TP], F32)
            dmc = cp.tile([128, NH], F32)
            dmb = cp.tile([128, NH * B], F32)
            for j in range(wscan):
                s = slice(j * NG * TP, (j + 1) * NG * TP)
                nc.sync.dma_start(dmCat[:, s], dmcat_d[:])
                nc.sync.dma_start(dsCat[:, s], dscat_d[:])
            nc.sync.dma_start(dmc[:], dmc_d[:])
            nc.sync.dma_start(dmb[:], dmb_d[:])

            # phase-2 state P = E/vth
            P = cp.tile([128, NH * B], F32)
            Pv = P[:].rearrange("p (h b) -> p h b", h=NH)

            if 1 not in phases:
                # phase-2-only benchmark mode: R is never written by phase 1
                nc.vector.memset(R[:], 0.1)

            def emit_body():
                if psplit > 0:
                    nc.vector.memset(P[:, : psplit * B], 0.0)
                if psplit < NH:
                    nc.gpsimd.memset(P[:, psplit * B :], 0.0)
                if 1 in phases:
                    emit_phase1()
                if 2 in phases:
                    emit_phase2()

            def emit_evict(xCat, i0):
                for j in range(wscan):
                    i = i0 + j
                    b0 = i * NB
                    W = NG * TP
                    x4 = xCat[:, j * W : (j + 1) * W].rearrange(
                        "p (h bl t) -> p h bl t", h=NH, bl=NB, t=TP
                    )
                    outv = Rv[:, :, :, b0 : b0 + NB].rearrange(
                        "p t h b -> p h b t"
                    )
                    if i in pool_evict:
                        nc.gpsimd.tensor_scalar(
                            outv, x4[:, :, :, 0:T], -1.0, None, op0=ALU.add
                        )
                    else:
                        nc.scalar.activation(
                            outv, x4[:, :, :, 0:T], COPY, -1.0
                        )

            def emit_phase1():
                W = NG * TP
                prev = None
                for i0 in range(0, NBAT, wscan):
                    xCat = xp.tile([128, wscan * W], F32, tag="xCat")
                    for j in range(wscan):
                        nc.sync.dma_start(
                            xCat[:, j * W : (j + 1) * W], x_d[:, i0 + j]
                        )
                    # two-pole cascade, scans in place over the x tile
                    nc.vector.tensor_tensor_scan(
                        xCat[:], xCat[:], dmCat[:], 0.0, op0=ALU.add, op1=ALU.mult
                    )
                    if sreorder:
                        # scan1(i+1) separates the scan1(i)->scan2(i) RAW pair
                        if prev is not None:
                            pCat, p0 = prev
                            nc.vector.tensor_tensor_scan(
                                pCat[:], pCat[:], dsCat[:], 0.0,
                                op0=ALU.add, op1=ALU.mult
                            )
                            emit_evict(pCat, p0)
                        prev = (xCat, i0)
                    else:
                        nc.vector.tensor_tensor_scan(
                            xCat[:], xCat[:], dsCat[:], 0.0,
                            op0=ALU.add, op1=ALU.mult
                        )
                        emit_evict(xCat, i0)
                if sreorder and prev is not None:
                    pCat, p0 = prev
                    nc.vector.tensor_tensor_scan(
                        pCat[:], pCat[:], dsCat[:], 0.0, op0=ALU.add, op1=ALU.mult
                    )
                    emit_evict(pCat, p0)

            # phase 2: o_t = (r''_t > P);  P_h = dm_h*P_h + o_h
            def emit_phase2():
                O = None
                for t in range(T):
                    if t % tblk == 0:
                        O = op.tile([128, tblk * NH * B], U8, tag="O")
                        Ov = O[:].rearrange(
                            "p (q h b) -> p q h b", q=tblk, h=NH, b=B
                        )
                    q = t % tblk
                    if p2hil:
                        # two interleaved h-pair chains with narrow stt
                        # updates; every RAW link separated by the other chain
                        for c in range(2):
                            hs = slice(2 * c, 2 * c + 2)
                            nc.vector.tensor_tensor(
                                Ov[:, q, hs], Rv[:, t, hs],
                                Pv[:, hs], op=ALU.is_gt,
                            )
                        if t < T - 1:
                            for hh in (0, 2, 1, 3):
                                nc.vector.scalar_tensor_tensor(
                                    Pv[:, hh, :], Pv[:, hh, :],
                                    dmc[:, hh : hh + 1], Ov[:, q, hh, :],
                                    op0=ALU.mult, op1=ALU.add,
                                )
                    elif p2il:
                        # interleaved independent b-range chains: every RAW
                        # link has an independent instruction between it, so
                        # pipeline drains overlap (the chain is latency-bound)
                        nchain = p2il
                        bs = B // nchain
                        dmbv = dmb[:].rearrange("p (h b) -> p h b", h=NH)
                        for c in range(nchain):
                            cb = slice(c * bs, (c + 1) * bs)
                            nc.vector.tensor_tensor(
                                Ov[:, q, :, cb], Rv[:, t, :, cb],
                                Pv[:, :, cb], op=ALU.is_gt,
                            )
                        if t < T - 1:
                            for c in range(nchain):
                                cb = slice(c * bs, (c + 1) * bs)
                                nc.vector.tensor_tensor(
                                    Pv[:, :, cb], Pv[:, :, cb],
                                    dmbv[:, :, cb], op=ALU.mult,
                                )
                            for c in range(nchain):
                                cb = slice(c * bs, (c + 1) * bs)
                                nc.vector.tensor_tensor(
                                    Pv[:, :, cb], Pv[:, :, cb],
                                    Ov[:, q, :, cb], op=ALU.add,
                                )
                    else:
                        if psplit > 0:
                            nc.vector.tensor_tensor(
                                Ov[:, q, 0:psplit], Rv[:, t, 0:psplit],
                                Pv[:, 0:psplit], op=ALU.is_gt,
                            )
                        if psplit < NH:
                            nc.gpsimd.tensor_tensor(
                                Ov[:, q, psplit:NH], Rv[:, t, psplit:NH],
                                Pv[:, psplit:NH], op=ALU.is_gt,
                            )
                        if t < T - 1:
                            if p2merge:
                                # P *= dm (broadcast tensor); P += o
                                nc.vector.tensor_tensor(
                                    P[:], P[:], dmb[:], op=ALU.mult
                                )
                                nc.vector.tensor_tensor(
                                    P[:], P[:], Ov[:, q], op=ALU.add
                                )
                            else:
                                for h in range(NH):
                                    eng = nc.vector if h < psplit else nc.gpsimd
                                    eng.scalar_tensor_tensor(
                                        Pv[:, h, :], Pv[:, h, :], dmc[:, h : h + 1],
                                        Ov[:, q, h, :], op0=ALU.mult, op1=ALU.add,
                                    )
                    if (t + 1) % tblk == 0:
                        s0 = (t + 1 - tblk) * NH * B
                        s1 = (t + 1) * NH * B
                        nc.sync.dma_start(out_d[:, s0:s1], O[:])

            if rep == 1:
                emit_body()
            else:
                # four bodies per hardware-loop iteration: quarters the
                # all-engine barrier + semaphore-reset overhead per body
                assert rep % 4 == 0
                with tc.For_i(0, rep // 4):
                    for _ in range(4):
                        emit_body()
    nc.finalize()
    return nc


def make_in_maps(x, decay_m, decay_s, vth):
    """Shard + host-side pre-scale/repack of x into the padded scan layout,
    plus the precomputed per-core scan decay tensors."""
    in_maps = []
    x = np.asarray(x, dtype=np.float32)
    dm_f = np.asarray(decay_m, dtype=np.float64)
    ds_f = np.asarray(decay_s, dtype=np.float64)
    vth_f = np.asarray(vth, dtype=np.float64)
    c_all = ((dm_f - ds_f) / (dm_f * ds_f * vth_f)).astype(np.float32)
    for c in range(NCORES):
        sl = slice(c * NLOC, (c + 1) * NLOC)
        xs = x[:, sl, :] * c_all[sl][None, :, None]
        # [B, NH, 128, T] -> [128, B, NH, T] -> padded [128, NBAT, NH, NB, TP]
        xv = xs.reshape(B, NH, 128, T).transpose(2, 0, 1, 3)
        A = np.zeros((128, NBAT, NH, NB, TP), np.float32)
        A[:, :, :, :, :T] = (
            xv.reshape(128, NBAT, NB, NH, T).transpose(0, 1, 3, 2, 4)
        )
        # decay tensors in the same (h, bl, t) group layout, 0 at separators
        dmv = np.asarray(decay_m[sl], dtype=np.float32).reshape(NH, 128)
        dsv = np.asarray(decay_s[sl], dtype=np.float32).reshape(NH, 128)
        dmcat = np.zeros((128, NH, NB, TP), np.float32)
        dscat = np.zeros((128, NH, NB, TP), np.float32)
        dmcat[:, :, :, :T] = dmv.T[:, :, None, None]
        dscat[:, :, :, :T] = dsv.T[:, :, None, None]
        in_maps.append(
            {
                "x": A.reshape(128, NBAT, NG * TP),
                "dmcat": dmcat.reshape(128, NG * TP),
                "dscat": dscat.reshape(128, NG * TP),
                "dmc": np.ascontiguousarray(dmv.T),
                "dmb": np.ascontiguousarray(
                    np.repeat(dmv.T, B, axis=1).reshape(128, NH * B)
                ),
            }
        )
    return in_maps


def kernel(x, decay_m, decay_s, vth):
    global _cached_program, LAST_RESULTS
    if _cached_program is None:
        _cached_program = build_program()
    nc = _cached_program

    in_maps = make_in_maps(x, decay_m, decay_s, vth)
    res = run_bass_kernel_spmd(nc, in_maps, core_ids=list(range(NCORES)))
    LAST_RESULTS = res
    out = np.empty((B, N, T), np.float32)
    for c in range(NCORES):
        out[:, c * NLOC : (c + 1) * NLOC, :] = unshard_core(
            res.results[c]["out"]
        )
    return out


def unshard_core(arr):
    """[128, T*NH*B] core output (uint8 or f32) -> [B, NLOC, T] f32."""
    r = np.asarray(arr).reshape(128, T, NH, B)
    # out[b, h*128+p, t] = r[p, t, h, b]
    return np.ascontiguousarray(
        r.transpose(3, 2, 0, 1).reshape(B, NLOC, T), dtype=np.float32
    )


# revision 18
# speedup vs baseline: 1.0004x; 1.0004x over previous
"""LIF spiking-neuron forward kernel for Trainium2 (8 NeuronCores, data-parallel
over neurons).

For x[B,N,T] and per-neuron params decay_m/decay_s/vth[N]:
    M_t = dm*(M_{t-1} + x_t);  S_t = ds*(S_{t-1} + x_t)
    E_t = dm*E_{t-1} + vth*o_{t-1}
    u_t = M_t - S_t - E_t - vth;  o_t = (u_t > 0)
returns the spike train o[B,N,T] (f32).

Per core (512 neurons = 4 chunks of 128 partitions).  Design notes:

  cascade:  M_t - S_t = (dm-ds) * H_t where H = scan_ds(scan_dm(x)) is the
            two-pole cascade of first-order scans (partial fractions of
            dm*z/((z-dm)(z-ds)) - ds*z/((z-dm)(z-ds)); the numerator
            difference is exactly (dm-ds)).  With the scan form
            state' = (state + x)*d the cascade yields dm*ds*H, so the host
            pre-scales x by c = (dm-ds)/(dm*ds*vth) per neuron, making
            r''_t = (M_t - S_t)/vth - 1 = cascade(c*x)_t - 1.
            This kills the former M-S subtract pass entirely and removes the
            per-partition 1/vth scale from the eviction.

  phase 1:  per b-batch (4 b's, all t): DMA x in, scan1 (dm) then scan2 (ds)
            in place on DVE (the HW scan runs at ~2.1ns/col = 2 cyc/elem,
            ~4.4us per 2064-col batch; this is the phase-1 floor), then one
            ScalarE activation evicts the batch into the big R tile with
            bias=-1 (layout (t, h, b), contiguous per t).  Evictions and DMA
            hide completely under the scans.

  phase 2:  normalized threshold recurrence (P = E/vth):
               o_t = (r''_t > P_t);  P_{t+1} = dm*P_t + o_t
            The chain is DVE-latency-bound (~213ns/instruction incl. drain),
            so it runs as TWO interLEAVED independent h-pair chains (P2HIL):
            per t: is_gt(h01), is_gt(h23), stt(h0), stt(h2), stt(h1), stt(h3)
            - every RAW link has an independent instruction between it, which
            overlaps the write-to-read pipeline drains (~1.28us/t vs 1.46us/t
            for the naive 5-instruction order).
            o is written as uint8 (exact 0/1) into a small per-t-block O tile
            and DMA'd out as uint8 (4x less output traffic); the stt P-update
            reads o back with uint8->f32 upcast.
            NOTE: the Pool engine only supports memset/copy/add/sub/mult/
            tensor_scalar (no comparisons/scans/stt - walrus rejects them),
            and is ~2x slower per column than DVE with ~250ns/instruction
            overhead, so phase 2 cannot be split across engines.

  host:     pre-scales/pads x into the scan layout and precomputes the scan
            decay tensors (dmcat/dscat with zero separator columns) so the
            device does no setup work; converts the uint8 spike train back to
            f32 on return.  Rounding differs from the reference by ~1e-6,
            flipping O(1) borderline spikes out of 33.5M (tolerance 2e-2).

  measured (HW, per-exec, 8 cores): phase 1 ~154us + phase 2 ~164us
  = ~299us total, vs 372us for the previous all-f32 subtract-based design.
"""

import numpy as np

import concourse.bacc as bacc
import concourse.bass as bass
import concourse.mybir as mybir
import concourse.tile as tile
from concourse.bass_utils import run_bass_kernel_spmd

F32 = mybir.dt.float32
U8 = mybir.dt.uint8
ALU = mybir.AluOpType
COPY = mybir.ActivationFunctionType.Copy

B, N, T = 64, 4096, 128
NCORES = 8
NLOC = N // NCORES          # 512 neurons per core
NH = NLOC // 128            # 4 neuron chunks of 128 (partition dim)
NB = 4                      # batch of b's per scan instruction
NBAT = B // NB              # 16 scan batches
NG = NB * NH                # 16 groups per scan batch, ordered h-major
TP = T + 1                  # per-group pitch in scan layout (sep column)
TBLK = 8                    # t-block size for the overlapped output DMA

# tunables (validated on HW).  NOTE: the Pool engine only supports
# memset/copy/add/sub/mult/tensor_scalar — no comparisons, no scans, no
# scalar_tensor_tensor — so phase 2 and the scans must stay on DVE.
PSPLIT = 4                          # chunks [0,PSPLIT) on DVE, rest on Pool
POOL_SCAN2 = frozenset()            # batches whose scan2 runs on Pool
P2MERGE = False                     # 3-instr/t phase 2 (wide tt) vs 5-instr
P2IL = 0                            # >0: interleave P2IL b-half chains
P2HIL = True                        # interleave 2 h-pair chains (stt updates)
WSCAN = 1                           # batches per scan instruction
SREORDER = False                    # software-pipeline scan1(i+1) before scan2(i)
POOL_EVICT = frozenset()            # batches evicted on Pool (ts add -1)


LAST_RESULTS = None

_cached_program = None


def build_program(rep: int = 1, psplit: int = PSPLIT,
                  pool_scan2=POOL_SCAN2, phases=(1, 2), tblk: int = TBLK,
                  hsep: bool = False, xbufs: int = 3,
                  p2merge: bool = P2MERGE, p2il: int = P2IL,
                  p2hil: bool = P2HIL, wscan: int = WSCAN,
                  sreorder: bool = SREORDER,
                  pool_evict=POOL_EVICT) -> bass.Bass:
    """rep=1 is the production kernel.  rep>1 wraps the whole computation in
    a hardware loop (tc.For_i) that re-runs it `rep` times per NEFF
    execution - used by test.py to amortize per-dispatch overhead out of the
    per-execution timing (each iteration redoes all DMA + compute)."""
    nc = bacc.Bacc(None, target_bir_lowering=False)
    # x pre-scaled by c[n] and pre-padded on host into the scan layout:
    # [128, NBAT, NG*TP], group g = h*NB + bl, b = i*NB + bl, n = h*128 + p.
    x_d = nc.declare_dram_parameter("x", [128, NBAT, NG * TP], F32, isOutput=False)
    dmcat_d = nc.declare_dram_parameter("dmcat", [128, NG * TP], F32, isOutput=False)
    dscat_d = nc.declare_dram_parameter("dscat", [128, NG * TP], F32, isOutput=False)
    dmc_d = nc.declare_dram_parameter("dmc", [128, NH], F32, isOutput=False)
    need_dmb = p2merge or p2il
    dmb_d = (
        nc.declare_dram_parameter("dmb", [128, NH * B], F32, isOutput=False)
        if need_dmb else None
    )
    # out[p, t*NH*B + h*B + b] = o[b, h*128+p, t] as uint8; host converts.
    out_d = nc.declare_dram_parameter("out", [128, T * NH * B], U8, isOutput=True)

    with tile.TileContext(nc) as tc:
        with (
            tc.tile_pool(name="big", bufs=1) as bigp,
            tc.tile_pool(name="xin", bufs=xbufs) as xp,
            tc.tile_pool(name="oout", bufs=2) as op,
            tc.tile_pool(name="const", bufs=1) as cp,
        ):
            # R: r'' in layout (t, h, b): f = t*(NH*B) + h*B + b
            R = bigp.tile([128, T * NH * B], F32)
            Rv = R[:].rearrange("p (t h b) -> p t h b", t=T, h=NH, b=B)

            dmCat = cp.tile([128, wscan * NG * TP], F32)
            dsCat = cp.tile([128, wscan * NG * TP], F32)
            dmc = cp.tile([128, NH], F32)
            if need_dmb:
                dmb = cp.tile([128, NH * B], F32)
            else:
                dmb = None
            # issue const loads from the Act engine's DGE so they don't
            # queue ahead of batch 0's x DMA on the SP queue
            for j in range(wscan):
                s = slice(j * NG * TP, (j + 1) * NG * TP)
                nc.scalar.dma_start(dmCat[:, s], dmcat_d[:])
                nc.scalar.dma_start(dsCat[:, s], dscat_d[:])
            nc.scalar.dma_start(dmc[:], dmc_d[:])
            if need_dmb:
                nc.scalar.dma_start(dmb[:], dmb_d[:])

            # phase-2 state P = E/vth
            P = cp.tile([128, NH * B], F32)
            Pv = P[:].rearrange("p (h b) -> p h b", h=NH)

            if 1 not in phases:
                # phase-2-only benchmark mode: R is never written by phase 1
                nc.vector.memset(R[:], 0.1)

            def emit_body():
                if psplit > 0:
                    nc.vector.memset(P[:, : psplit * B], 0.0)
                if psplit < NH:
                    nc.gpsimd.memset(P[:, psplit * B :], 0.0)
                if 1 in phases:
                    emit_phase1()
                if 2 in phases:
                    emit_phase2()

            def emit_evict(xCat, i0):
                for j in range(wscan):
                    i = i0 + j
                    b0 = i * NB
                    W = NG * TP
                    x4 = xCat[:, j * W : (j + 1) * W].rearrange(
                        "p (h bl t) -> p h bl t", h=NH, bl=NB, t=TP
                    )
                    outv = Rv[:, :, :, b0 : b0 + NB].rearrange(
                        "p t h b -> p h b t"
                    )
                    if i in pool_evict:
                        nc.gpsimd.tensor_scalar(
                            outv, x4[:, :, :, 0:T], -1.0, None, op0=ALU.add
                        )
                    else:
                        nc.scalar.activation(
                            outv, x4[:, :, :, 0:T], COPY, -1.0
                        )

            def emit_phase1():
                W = NG * TP
                prev = None
                for i0 in range(0, NBAT, wscan):
                    xCat = xp.tile([128, wscan * W], F32, tag="xCat")
                    for j in range(wscan):
                        nc.sync.dma_start(
                            xCat[:, j * W : (j + 1) * W], x_d[:, i0 + j]
                        )
                    # two-pole cascade, scans in place over the x tile
                    nc.vector.tensor_tensor_scan(
                        xCat[:], xCat[:], dmCat[:], 0.0, op0=ALU.add, op1=ALU.mult
                    )
                    if sreorder:
                        # scan1(i+1) separates the scan1(i)->scan2(i) RAW pair
                        if prev is not None:
                            pCat, p0 = prev
                            nc.vector.tensor_tensor_scan(
                                pCat[:], pCat[:], dsCat[:], 0.0,
                                op0=ALU.add, op1=ALU.mult
                            )
                            emit_evict(pCat, p0)
                        prev = (xCat, i0)
                    else:
                        nc.vector.tensor_tensor_scan(
                            xCat[:], xCat[:], dsCat[:], 0.0,
                            op0=ALU.add, op1=ALU.mult
                        )
                        emit_evict(xCat, i0)
                if sreorder and prev is not None:
                    pCat, p0 = prev
                    nc.vector.tensor_tensor_scan(
                        pCat[:], pCat[:], dsCat[:], 0.0, op0=ALU.add, op1=ALU.mult
                    )
                    emit_evict(pCat, p0)

            # phase 2: o_t = (r''_t > P);  P_h = dm_h*P_h + o_h
            def emit_phase2():
                O = None
                for t in range(T):
                    if t % tblk == 0:
                        O = op.tile([128, tblk * NH * B], U8, tag="O")
                        Ov = O[:].rearrange(
                            "p (q h b) -> p q h b", q=tblk, h=NH, b=B
                        )
                    q = t % tblk
                    if p2hil:
                        # two interleaved h-pair chains with narrow stt
                        # updates; every RAW link separated by the other chain
                        for c in range(2):
                            hs = slice(2 * c, 2 * c + 2)
                            nc.vector.tensor_tensor(
                                Ov[:, q, hs], Rv[:, t, hs],
                                Pv[:, hs], op=ALU.is_gt,
                            )
                        if t < T - 1:
                            for hh in (0, 2, 1, 3):
                                nc.vector.scalar_tensor_tensor(
                                    Pv[:, hh, :], Pv[:, hh, :],
                                    dmc[:, hh : hh + 1], Ov[:, q, hh, :],
                                    op0=ALU.mult, op1=ALU.add,
                                )
                    elif p2il:
                        # interleaved independent b-range chains: every RAW
                        # link has an independent instruction between it, so
                        # pipeline drains overlap (the chain is latency-bound)
                        nchain = p2il
                        bs = B // nchain
                        dmbv = dmb[:].rearrange("p (h b) -> p h b", h=NH)
                        for c in range(nchain):
                            cb = slice(c * bs, (c + 1) * bs)
                            nc.vector.tensor_tensor(
                                Ov[:, q, :, cb], Rv[:, t, :, cb],
                                Pv[:, :, cb], op=ALU.is_gt,
                            )
                        if t < T - 1:
                            for c in range(nchain):
                                cb = slice(c * bs, (c + 1) * bs)
                                nc.vector.tensor_tensor(
                                    Pv[:, :, cb], Pv[:, :, cb],
                                    dmbv[:, :, cb], op=ALU.mult,
                                )
                            for c in range(nchain):
                                cb = slice(c * bs, (c + 1) * bs)
                                nc.vector.tensor_tensor(
                                    Pv[:, :, cb], Pv[:, :, cb],
                                    Ov[:, q, :, cb], op=ALU.add,
                                )
                    else:
                        if psplit > 0:
                            nc.vector.tensor_tensor(
                                Ov[:, q, 0:psplit], Rv[:, t, 0:psplit],
                                Pv[:, 0:psplit], op=ALU.is_gt,
                            )
                        if psplit < NH:
                            nc.gpsimd.tensor_tensor(
                                Ov[:, q, psplit:NH], Rv[:, t, psplit:NH],
                                Pv[:, psplit:NH], op=ALU.is_gt,
                            )
                        if t < T - 1:
                            if p2merge:
                                # P *= dm (broadcast tensor); P += o
                                nc.vector.tensor_tensor(
                                    P[:], P[:], dmb[:], op=ALU.mult
                                )
                                nc.vector.tensor_tensor(
                                    P[:], P[:], Ov[:, q], op=ALU.add
                                )
                            else:
                                for h in range(NH):
                                    eng = nc.vector if h < psplit else nc.gpsimd
                                    eng.scalar_tensor_tensor(
                                        Pv[:, h, :], Pv[:, h, :], dmc[:, h : h + 1],
                                        Ov[:, q, h, :], op0=ALU.mult, op1=ALU.add,
                                    )
                    if (t + 1) % tblk == 0:
                        s0 = (t + 1 - tblk) * NH * B
                        s1 = (t + 1) * NH * B
                        nc.sync.dma_start(out_d[:, s0:s1], O[:])

            if rep == 1:
                emit_body()
            else:
                # four bodies per hardware-loop iteration: quarters the
                # all-engine barrier + semaphore-reset overhead per body
                assert rep % 4 == 0
                with tc.For_i(0, rep // 4):
                    for _ in range(4):
                        emit_body()
    nc.finalize()
    return nc


def make_in_maps(x, decay_m, decay_s, vth):
    """Shard + host-side pre-scale/repack of x into the padded scan layout,
    plus the precomputed per-core scan decay tensors."""
    in_maps = []
    x = np.asarray(x, dtype=np.float32)
    dm_f = np.asarray(decay_m, dtype=np.float64)
    ds_f = np.asarray(decay_s, dtype=np.float64)
    vth_f = np.asarray(vth, dtype=np.float64)
    c_all = ((dm_f - ds_f) / (dm_f * ds_f * vth_f)).astype(np.float32)
    for c in range(NCORES):
        sl = slice(c * NLOC, (c + 1) * NLOC)
        xs = x[:, sl, :] * c_all[sl][None, :, None]
        # [B, NH, 128, T] -> [128, B, NH, T] -> padded [128, NBAT, NH, NB, TP]
        xv = xs.reshape(B, NH, 128, T).transpose(2, 0, 1, 3)
        A = np.zeros((128, NBAT, NH, NB, TP), np.float32)
        A[:, :, :, :, :T] = (
            xv.reshape(128, NBAT, NB, NH, T).transpose(0, 1, 3, 2, 4)
        )
        # decay tensors in the same (h, bl, t) group layout, 0 at separators
        dmv = np.asarray(decay_m[sl], dtype=np.float32).reshape(NH, 128)
        dsv = np.asarray(decay_s[sl], dtype=np.float32).reshape(NH, 128)
        dmcat = np.zeros((128, NH, NB, TP), np.float32)
        dscat = np.zeros((128, NH, NB, TP), np.float32)
        dmcat[:, :, :, :T] = dmv.T[:, :, None, None]
        dscat[:, :, :, :T] = dsv.T[:, :, None, None]
        in_maps.append(
            {
                "x": A.reshape(128, NBAT, NG * TP),
                "dmcat": dmcat.reshape(128, NG * TP),
                "dscat": dscat.reshape(128, NG * TP),
                "dmc": np.ascontiguousarray(dmv.T),
                "dmb": np.ascontiguousarray(
                    np.repeat(dmv.T, B, axis=1).reshape(128, NH * B)
                ),
            }
        )
    return in_maps


def kernel(x, decay_m, decay_s, vth):
    global _cached_program, LAST_RESULTS
    if _cached_program is None:
        _cached_program = build_program()
    nc = _cached_program

    in_maps = make_in_maps(x, decay_m, decay_s, vth)
    res = run_bass_kernel_spmd(nc, in_maps, core_ids=list(range(NCORES)))
    LAST_RESULTS = res
    out = np.empty((B, N, T), np.float32)
    for c in range(NCORES):
        out[:, c * NLOC : (c + 1) * NLOC, :] = unshard_core(
            res.results[c]["out"]
        )
    return out


def unshard_core(arr):
    """[128, T*NH*B] core output (uint8 or f32) -> [B, NLOC, T] f32."""
    r = np.asarray(arr).reshape(128, T, NH, B)
    # out[b, h*128+p, t] = r[p, t, h, b]
    return np.ascontiguousarray(
        r.transpose(3, 2, 0, 1).reshape(B, NLOC, T), dtype=np.float32
    )
